# revision 20
# baseline (speedup 1.0000x reference)
"""GPT-2 causal attention block (S=4096, D=768, H=12) on 8 TRN2 NeuronCores.

Sharding: queries interleaved mod-8 (core c owns q = 8*t + c) -> every core
runs the identical SPMD graph (uniform causal work), per-core differences are
input data only (per-core causal masks, output row mapping).
K/V projection is feature-sharded (96 of 768+768 K/V features per core over
all 4096 keys), followed by two AllGathers split by key halves so attention
on the first 2048 keys starts while the second AllGather is in flight.

Device algorithm per core (bf16 compute, f32 accumulate):
  A: dc-pipelined projections straight off the token DMAs:
     A1w1 K^T[96, 0:2048], A2w1 V[0:2048, 96]  -> AllGather #1 (issued ~25us)
     A1w2 K^T[96, 2048:], A2w2 V[2048:, 96]    -> AllGather #2
     A3 Q^T local [768,512] (overlaps the collectives)
  C: load K^T full [768,4096] to SBUF per key-half as gathers land
  D: per q-tile T (128 logical rows), per group g of 8 k-chunks:
     scores S^T [128k,128q] per chunk -> PSUM staging [128,1024]
     one wide exp -> P^T bf16 SBUF; causal mask multiply on diagonal groups;
     den rides the PV matmul as a ones column in lhsT -> O^T accum
  E: den reciprocal, broadcast matmul, normalize O^T -> A^T
  F: c_proj (A^T as lhsT) + bias via K=1 ones-matmul -> out [512,768] f32
"""
import numpy as np
import ml_dtypes

import concourse.bass as bass
import concourse.bacc as bacc
import concourse.mybir as mybir
import concourse.tile as tile
from concourse.bass_utils import run_bass_kernel_spmd

BF16 = mybir.dt.bfloat16
F32 = mybir.dt.float32
AF = mybir.ActivationFunctionType

S = 4096          # sequence
D = 768           # model dim
H = 12            # heads
HD = 64           # head dim
NC = 8            # cores
QL = S // NC      # 512 queries per core
NKC = S // 128    # 32 k-chunks
KVF = 2 * D // NC  # 192 kv features per core (96 K + 96 V)
KF = KVF // 2      # 96
SH = S // 2        # 2048 keys per gather half
KH = KF * SH       # K^T half elems per core
VH = SH * KF       # V half elems per core

TRACE = False  # test.py sets True for neuron-profile timing

_CACHE = {}


def build_bass():
    nc = bacc.Bacc(
        "TRN2", target_bir_lowering=False, debug=False, num_devices=NC
    )

    # ---- DRAM parameters (per-core inputs; all bf16 except out) ----
    # weight matrices are host-packed partition-major so every per-partition
    # DMA read is one contiguous run
    tok_t = nc.declare_dram_parameter("tok_t", [D, S], BF16, isOutput=False)
    tok_q = nc.declare_dram_parameter("tok_q", [D, QL], BF16, isOutput=False)
    w_q = nc.declare_dram_parameter("w_q", [128, 6 * D], BF16, isOutput=False)
    b_q = nc.declare_dram_parameter("b_q", [128, 6], F32, isOutput=False)
    w_kv = nc.declare_dram_parameter("w_kv", [128, 6 * KVF], BF16, isOutput=False)
    b_k = nc.declare_dram_parameter("b_k", [128, 1], F32, isOutput=False)
    b_v = nc.declare_dram_parameter("b_v", [1, KF], BF16, isOutput=False)
    w_p = nc.declare_dram_parameter("w_p", [128, 6 * D], BF16, isOutput=False)
    b_p = nc.declare_dram_parameter("b_p", [1, D], BF16, isOutput=False)
    maskp = nc.declare_dram_parameter("maskp", [128, 8, 256], BF16, isOutput=False)
    onesBPp = nc.declare_dram_parameter("onesBPp", [128, 64], F32, isOutput=False)
    ones128p = nc.declare_dram_parameter("ones128p", [128, 1], F32, isOutput=False)
    ones1p = nc.declare_dram_parameter("ones1p", [1, 128], BF16, isOutput=False)
    out = nc.declare_dram_parameter("out", [QL, D], F32, isOutput=True)

    # internal DRAM: collective bounce buffers, one per key half
    kv1_bounce = nc.dram_tensor("kv1_bounce", [KH + VH], BF16)
    kv2_bounce = nc.dram_tensor("kv2_bounce", [KH + VH], BF16)
    kv1_gath = nc.dram_tensor("kv1_gath", [NC, KH + VH], BF16, addr_space="Shared")
    kv2_gath = nc.dram_tensor("kv2_gath", [NC, KH + VH], BF16, addr_space="Shared")

    kb_k1 = kv1_bounce[0:KH].rearrange("(f s) -> f s", f=KF)    # [96, 2048]
    kb_v1 = kv1_bounce[KH:].rearrange("(s f) -> s f", s=SH)     # [2048, 96]
    kb_k2 = kv2_bounce[0:KH].rearrange("(f s) -> f s", f=KF)
    kb_v2 = kv2_bounce[KH:].rearrange("(s f) -> s f", s=SH)

    with tile.TileContext(nc) as tc:
        with (
            tc.tile_pool(name="qt", bufs=1) as qt_pool,
            tc.tile_pool(name="kt", bufs=1) as kt_pool,
            tc.tile_pool(name="at", bufs=1) as at_pool,
            tc.tile_pool(name="const", bufs=1) as const_pool,
        ):
            # ---------- persistent SBUF ----------
            qt = qt_pool.tile([128, 6, QL], BF16)      # Q^T  [feat, q]
            kts = [kt_pool.tile([128, S], BF16, tag=f'kt{i}', name=f'ktf{i}') for i in range(6)]
            aT = at_pool.tile([128, 6, QL], BF16)      # normalized attn out^T
            mask_sb = const_pool.tile([128, 8, 256], BF16)
            onesBP = const_pool.tile([128, 64], F32)
            ones128 = const_pool.tile([128, 1], F32)
            ones1 = const_pool.tile([1, 128], BF16)
            zrow = const_pool.tile([1, 512], BF16)
            nc.vector.memset(zrow[:], 0.0)
            bq_sb = const_pool.tile([128, 6], F32)
            bk_sb = const_pool.tile([128, 1], F32)
            bv_sb = const_pool.tile([1, KF], BF16)
            bp_sb = const_pool.tile([1, D], BF16)
            wp_sb = const_pool.tile([128, 6, D], BF16)

            nc.sync.dma_start(out=bq_sb[:], in_=b_q[:])
            nc.sync.dma_start(out=bk_sb[:], in_=b_k[:])
            nc.sync.dma_start(out=bv_sb[:], in_=b_v[:])
            nc.sync.dma_start(out=ones1[:], in_=ones1p[:])

            # DVE pre-touch of DVE-read consts: TensorScalar/TensorTensor ISA
            # structs carry only ONE sync wait, so the DMA deps must already
            # be covered by the DVE engine clock before first real use.
            warm = const_pool.tile([128, 4], F32)
            warme = const_pool.tile([128, 4], BF16)
            nc.vector.tensor_copy(warm[:, 0:1], bk_sb[:, 0:1])
            # preload the exp activation table set during phase A
            nc.scalar.activation(warme[:, :], warm[:, :], AF.Exp)
            nc.vector.tensor_copy(warm[:, 1:2], bq_sb[:, 0:1])

            # ---------- phase A: projections ----------
            with (
                tc.tile_pool(name="tokt", bufs=1) as tokt_pool,
                tc.tile_pool(name="wA", bufs=1) as wA_pool,
                tc.tile_pool(name="ktloc", bufs=1) as ktloc_pool,
                tc.tile_pool(name="vev", bufs=3) as vev_pool,
                tc.tile_pool(name="psA", bufs=4, space="PSUM") as psA,
            ):
                # weights and tokens are split across many dma_starts so the
                # 16 DMA engines run in parallel (one dma_start = one queue)
                wkv_sb = wA_pool.tile([128, 6, KVF], BF16)
                wkv_v = w_kv.rearrange("p (dc e) -> p dc e", e=KVF)
                for dc in range(6):
                    nc.sync.dma_start(out=wkv_sb[:, dc, :], in_=wkv_v[:, dc, :])
                tok_sb = tokt_pool.tile([128, 6, 4, 1024], BF16)
                tok_v = tok_t.rearrange("(dc p) (qq s) -> p dc qq s", p=128, s=1024)
                for qq in range(4):
                    for dc in range(6):
                        nc.sync.dma_start(
                            out=tok_sb[:, dc, qq, :], in_=tok_v[:, dc, qq, :]
                        )
                tokq_sb = tokt_pool.tile([128, 6, QL], BF16)
                nc.sync.dma_start(
                    out=tokq_sb[:], in_=tok_q.rearrange("(dc p) s -> p dc s", p=128)
                )
                wq_sb = wA_pool.tile([128, 6, D], BF16)
                wq_v = w_q.rearrange("p (dc e) -> p dc e", e=D)
                for dc in range(6):
                    nc.sync.dma_start(out=wq_sb[:, dc, :], in_=wq_v[:, dc, :])

                kt_loc = ktloc_pool.tile([128, 2, SH], BF16)

                def a1_wave(h):
                    # K^T local for keys [2048h, 2048h+2048), dc-outer so
                    # matmuls chase the token DMAs
                    accs = [
                        psA.tile([128, 512], F32, tag="psa", name=f"acc{h}_{i}")
                        for i in range(4)
                    ]
                    for dc in range(6):
                        for sc in range(4):
                            qq, s0 = 2 * h + sc // 2, 512 * (sc % 2)
                            nc.tensor.matmul(
                                accs[sc][0:KF, :],
                                lhsT=wkv_sb[:, dc, 0:KF],
                                rhs=tok_sb[:, dc, qq, s0:s0 + 512],
                                start=(dc == 0),
                                stop=(dc == 5),
                            )
                    for sc in range(4):
                        nc.vector.tensor_scalar_add(
                            kt_loc[0:KF, h, 512 * sc:512 * (sc + 1)],
                            accs[sc][0:KF, :],
                            bk_sb[0:KF, 0:1],
                        )

                def a2_wave(h, kb_v):
                    # V local [2048, 96] natural orientation for this key half
                    for st in range(16):
                        qq, s0 = 2 * h + st // 8, 128 * (st % 8)
                        ps = psA.tile([128, 512], F32, tag="psa")
                        for dc in range(6):
                            nc.tensor.matmul(
                                ps[:, 0:KF],
                                lhsT=tok_sb[:, dc, qq, s0:s0 + 128],
                                rhs=wkv_sb[:, dc, KF:KVF],
                                start=(dc == 0),
                                stop=False,
                            )
                        nc.tensor.matmul(
                            ps[:, 0:KF],
                            lhsT=ones1[:, :],
                            rhs=bv_sb[:, :],
                            start=False,
                            stop=True,
                        )
                        vev = vev_pool.tile([128, KF], BF16)
                        nc.vector.tensor_copy(vev[:], ps[:, 0:KF])
                        nc.sync.dma_start(
                            out=kb_v[128 * st:128 * (st + 1), :], in_=vev[:]
                        )

                a1_wave(0)
                nc.sync.dma_start(out=kb_k1[:, :], in_=kt_loc[0:KF, 0, :])
                a2_wave(0, kb_v1)
                nc.gpsimd.collective_compute(
                    "AllGather",
                    mybir.AluOpType.bypass,
                    replica_groups=[list(range(NC))],
                    ins=[kv1_bounce.ap().opt()],
                    outs=[kv1_gath.ap().opt()],
                )

                a1_wave(1)
                nc.sync.dma_start(out=kb_k2[:, :], in_=kt_loc[0:KF, 1, :])
                a2_wave(1, kb_v2)
                nc.gpsimd.collective_compute(
                    "AllGather",
                    mybir.AluOpType.bypass,
                    replica_groups=[list(range(NC))],
                    ins=[kv2_bounce.ap().opt()],
                    outs=[kv2_gath.ap().opt()],
                )

                wp_v = w_p.rearrange("p (dc e) -> p dc e", e=D)
                for dc in range(6):
                    nc.sync.dma_start(out=wp_sb[:, dc, :], in_=wp_v[:, dc, :])
                # deferred const loads (masks not needed until attention)
                nc.sync.dma_start(out=mask_sb[:], in_=maskp[:])
                nc.sync.dma_start(out=onesBP[:], in_=onesBPp[:])
                nc.sync.dma_start(out=ones128[:], in_=ones128p[:])
                nc.sync.dma_start(out=bp_sb[:], in_=b_p[:])
                nc.vector.tensor_copy(warm[:, 2:3], mask_sb[:, 0, 0:1])
                # A3: Q^T local [768, 512] from per-core tok_q input
                for jc in range(6):
                    ps = psA.tile([128, 512], F32, tag="psa")
                    for dc in range(6):
                        nc.tensor.matmul(
                            ps[:, :],
                            lhsT=wq_sb[:, dc, 128 * jc:128 * (jc + 1)],
                            rhs=tokq_sb[:, dc, :],
                            start=(dc == 0),
                            stop=(dc == 5),
                        )
                    nc.vector.tensor_scalar_add(
                        qt[:, jc, :], ps[:, :], bq_sb[:, jc:jc + 1]
                    )

            # ---------- phase C: load K^T full, per gathered key half ----------
            # half 2 waits on AllGather #2; issue it from the gpsimd queue so
            # it cannot head-of-line block the sync queue's V-staging DMAs
            for half, gath, eng in ((0, kv1_gath, nc.sync), (1, kv2_gath, nc.gpsimd)):
                for hp in range(6):
                    f0 = 128 * hp
                    r0, row0 = f0 // KF, f0 % KF
                    n0 = min(KF - row0, 128)
                    gk0 = gath[r0, 0:KH].rearrange("(f s) -> f s", f=KF)
                    eng.dma_start(
                        out=kts[hp][0:n0, SH * half:SH * (half + 1)],
                        in_=gk0[row0:row0 + n0, :],
                    )
                    if n0 < 128:
                        gk1 = gath[r0 + 1, 0:KH].rearrange("(f s) -> f s", f=KF)
                        eng.dma_start(
                            out=kts[hp][n0:128, SH * half:SH * (half + 1)],
                            in_=gk1[0:128 - n0, :],
                        )

            # ---------- phase D: attention ----------
            # P^T staging g-major [128, 4, 256]; den rides the PV matmul as a
            # ones column in lhsT (out row 64). O^T accumulates directly in
            # PSUM across groups (4 heads per pass, 3 passes per q-tile);
            # within a shared PSUM bank only the first matmul uses start=True
            # (start clears has_written for the whole bank), bank-mates'
            # first writes rely on overwrite-where-bit-unset.
            gv_all1 = kv1_gath[:, KH:].rearrange("r (s f) -> s r f", s=SH)
            gv_all2 = kv2_gath[:, KH:].rearrange("r (s f) -> s r f", s=SH)
            with (
                tc.tile_pool(name="vres", bufs=32) as vres_pool,
                tc.tile_pool(name="vstg", bufs=6) as vstg_pool,
                tc.tile_pool(name="phat", bufs=6) as phat_pool,
                tc.tile_pool(name="tmpo", bufs=4) as tmpo_pool,
                tc.tile_pool(name="denp", bufs=2) as denp_pool,
                tc.tile_pool(name="oev", bufs=3) as oev_pool,
                tc.tile_pool(name="psStage", bufs=2, space="PSUM") as psS,
                tc.tile_pool(name="psO", bufs=1, space="PSUM") as psO,
                tc.tile_pool(name="psPV", bufs=2, space="PSUM") as psPV,
            ):
                # V chunks are staged lazily inside the group loop:
                # DMA gathered [128, 8, 96] (192B runs) to SBUF, then one DVE
                # copy into the 65-strided per-head layout (col 65h+64 = 1).
                vts = {}

                def prep_chunk(ch):
                    vstg = vstg_pool.tile([128, NC, KF], BF16, tag="vstg")
                    if ch < 16:
                        gsrc = gv_all1[128 * ch:128 * (ch + 1), :, :]
                    else:
                        gsrc = gv_all2[128 * (ch - 16):128 * (ch - 15), :, :]
                    nc.sync.dma_start(out=vstg[:], in_=gsrc)
                    vt = vres_pool.tile([128, H * 65], BF16, tag="vres")
                    vtv = vt[:].rearrange("p (h w) -> p h w", w=65)
                    nc.vector.memset(vtv[:, :, 64:65], 1.0)
                    nc.vector.tensor_copy(
                        vtv[:, :, 0:64],
                        vstg[:].rearrange("p r f -> p (r f)"),
                    )
                    vts[ch] = vt

                def emit_front(TT, g, h):
                    hp, hh = h // 2, h % 2
                    g_rel = g - 4 * TT
                    masked = g_rel >= 0
                    q0 = max(0, 64 * g_rel)
                    mi = 4 * TT + g_rel
                    p0, p1 = 64 * hh, 64 * hh + 64
                    stg = psS.tile([128, 1024], F32, tag="stage")
                    for kc in range(4):
                        ch = 4 * g + kc
                        nc.tensor.matmul(
                            stg[:, 256 * kc + q0:256 * (kc + 1)],
                            lhsT=kts[hp][p0:p1, 128 * ch:128 * (ch + 1)],
                            rhs=qt[p0:p1, hp, 256 * TT + q0:256 * (TT + 1)],
                            start=True,
                            stop=True,
                        )
                    phat = phat_pool.tile([128, 4, 256], BF16, tag="phat")
                    stgv = stg[:].rearrange("p (g q) -> p g q", g=4)
                    nc.scalar.activation(
                        phat[:, :, q0:256],
                        stgv[:, :, q0:256],
                        AF.Exp,
                    )
                    if masked:
                        # only the 64-wide diagonal band [q0, q0+64) can be
                        # partially visible; chunk kc's boundary region is
                        # within [q0, q0+16(kc+1)) and everything beyond the
                        # band is fully visible
                        mv = mask_sb[:, mi, :].rearrange("p (g w) -> p g w", w=64)
                        nc.vector.tensor_mul(
                            phat[:, :, q0:q0 + 64],
                            phat[:, :, q0:q0 + 64],
                            mv[:, :, :],
                        )
                    return phat

                def zero_oacc(oaccP):
                    # one start=True matmul per PSUM bank covering BOTH head
                    # regions: writes zeros with has_written set, and every
                    # PV matmul's region overlaps it so the tile scheduler
                    # orders all accumulation after it
                    for b in range(2):
                        nc.tensor.matmul(
                            oaccP[0:65, 2 * b:2 * b + 2, :],
                            lhsT=ones1[0:1, 0:65],
                            rhs=zrow[:, :],
                            start=True,
                            stop=False,
                            skip_group_check=True,
                        )

                def emit_back(st):
                    TT, g, hh, h, glast, oaccP, phat = st
                    g_rel = g - 4 * TT
                    q0 = max(0, 64 * g_rel)
                    for kc in range(4):
                        ch = 4 * g + kc
                        nc.tensor.matmul(
                            oaccP[0:65, hh, q0:256],
                            lhsT=vts[ch][:, 65 * h:65 * h + 65],
                            rhs=phat[:, kc, q0:256],
                            start=False,
                            stop=(g == glast and kc == 3),
                            skip_group_check=True,
                        )

                def do_norm(TT, pp, oaccP):
                    # reciprocal_approx_fast silently misbehaves on a PSUM
                    # operand — bounce the denominator row to SBUF first
                    denrow = denp_pool.tile([1, 4, 256], F32, tag="denrow")
                    nc.vector.tensor_copy(
                        denrow[:].rearrange("p h q -> p (h q)"),
                        oaccP[64:65, :, :].rearrange("p h q -> p (h q)"),
                    )
                    rrow = denp_pool.tile([1, 4, 256], F32, tag="rrow")
                    nc.vector.reciprocal_approx_fast(
                        rrow[:].rearrange("p h q -> p (h q)"),
                        denrow[:].rearrange("p h q -> p (h q)"),
                    )
                    for hh in range(4):
                        h = 4 * pp + hh
                        hp = h // 2
                        bc = psPV.tile([128, 256], F32, tag="pv")
                        nc.tensor.matmul(
                            bc[0:64, :],
                            lhsT=onesBP[0:1, :],
                            rhs=rrow[:, hh, :],
                            start=True,
                            stop=True,
                        )
                        # bounce the broadcast to SBUF: TensorTensor may read
                        # at most one operand from PSUM (oaccP is the other)
                        bcs = tmpo_pool.tile([64, 256], F32, tag="bcs")
                        nc.vector.tensor_copy(bcs[:, :], bc[0:64, :])
                        if hh % 2 == 0:
                            nc.vector.tensor_mul(
                                aT[0:64, hp, 256 * TT:256 * (TT + 1)],
                                oaccP[0:64, hh, :],
                                bcs[:, :],
                            )
                        else:
                            tmp = tmpo_pool.tile([64, 256], BF16, tag="tmpo")
                            nc.vector.tensor_mul(
                                tmp[:, :], oaccP[0:64, hh, :], bcs[:, :]
                            )
                            nc.gpsimd.dma_start(
                                out=aT[64:128, hp, 256 * TT:256 * (TT + 1)],
                                in_=tmp[:, :],
                            )

                def emit_cproj(st):
                    for ec, ew in ((0, 512), (1, 256)):
                        ps = psPV.tile([128, 512], F32, tag="pv")
                        for dc in range(6):
                            nc.tensor.matmul(
                                ps[:, 0:ew],
                                lhsT=aT[:, dc, 128 * st:128 * (st + 1)],
                                rhs=wp_sb[:, dc, 512 * ec:512 * ec + ew],
                                start=(dc == 0),
                                stop=False,
                            )
                        nc.tensor.matmul(
                            ps[:, 0:ew],
                            lhsT=ones1[:, :],
                            rhs=bp_sb[:, 512 * ec:512 * ec + ew],
                            start=False,
                            stop=True,
                        )
                        oev = oev_pool.tile([128, 512], F32, tag="oev")
                        nc.vector.tensor_copy(oev[:, 0:ew], ps[:, 0:ew])
                        nc.gpsimd.dma_start(
                            out=out[128 * st:128 * (st + 1), 512 * ec:512 * ec + ew],
                            in_=oev[:, 0:ew],
                        )

                # stage V chunks for the first gathered key half upfront
                # (DVE is idle during the collectives); second half stays
                # lazy so its DVE copies can't head-of-line block the queue
                for ch in range(16):
                    prep_chunk(ch)

                for TT in range(2):
                    glast = 4 * (TT + 1) - 1
                    for pp in range(3):
                        oaccP = psO.tile([128, 4, 256], F32, tag="oaccP")
                        zero_oacc(oaccP)
                        pending = None
                        for g in range(4 * (TT + 1)):
                            for ch in range(4 * g, 4 * g + 4):
                                if ch not in vts:
                                    prep_chunk(ch)
                            for hh in range(4):
                                h = 4 * pp + hh
                                phat = emit_front(TT, g, h)
                                if pending is not None:
                                    emit_back(pending)
                                pending = (TT, g, hh, h, glast, oaccP, phat)
                        emit_back(pending)
                        do_norm(TT, pp, oaccP)
                    for st in (0, 1) if TT == 0 else (2, 3):
                        emit_cproj(st)

    nc.compile()
    return nc


def make_inputs_v2(tokens, c_attn_weight, c_attn_bias, c_proj_weight, c_proj_bias):
    bf = ml_dtypes.bfloat16
    tokens = np.asarray(tokens, np.float32)
    w = np.asarray(c_attn_weight, np.float32)
    b = np.asarray(c_attn_bias, np.float32)
    wp = np.asarray(c_proj_weight, np.float32)
    bp = np.asarray(c_proj_bias, np.float32)

    def pack_pmajor(m):
        # [768, E] -> [128, 6*E]: partition-major so each partition's DMA
        # read is one contiguous run
        e = m.shape[1]
        return np.ascontiguousarray(
            m.reshape(6, 128, e).transpose(1, 0, 2).reshape(128, 6 * e)
        )

    scale = 1.0 / np.sqrt(HD)
    tok_t_full = np.ascontiguousarray(tokens.T).astype(bf)          # [768, 4096]
    w_q = pack_pmajor(w[:, 0:D] * scale).astype(bf)                 # [128, 4608]
    b_q = (b[0:D] * scale).reshape(6, 128).T.copy().astype(np.float32)  # [128, 6]
    w_p = pack_pmajor(wp).astype(bf)                                # [128, 4608]
    b_p = bp.reshape(1, D).astype(bf)

    onesBP = np.ones((128, 64), np.float32)
    ones128 = np.ones((128, 1), np.float32)
    ones1 = np.ones((1, 128), bf)

    in_maps = []
    for c in range(NC):
        qcols = np.arange(QL) * NC + c
        tok_q = np.ascontiguousarray(tokens.T[:, qcols]).astype(bf)  # [768, 512]

        kf = slice(D + KF * c, D + KF * (c + 1))
        vf = slice(2 * D + KF * c, 2 * D + KF * (c + 1))
        w_kv = pack_pmajor(
            np.concatenate([w[:, kf], w[:, vf]], axis=1)
        ).astype(bf)                                                # [128, 1152]
        b_k = np.zeros((128, 1), np.float32)
        b_k[0:KF, 0] = b[kf]
        b_v = b[vf].reshape(1, KF).astype(bf)

        # 64-wide diagonal-band mask, mask[i, m, kc*64 + jj] where the band
        # covers q columns j = 64*gr + jj of diagonal group m = 4*TT + gr;
        # chunk ch = 4*g_abs + kc; q_abs = 2048*TT + 8*j + c. Everything
        # beyond the band is fully visible, below the band not computed.
        mask = np.zeros((128, 8, 256), np.float32)
        i = np.arange(128).reshape(128, 1, 1)
        kc = np.arange(4).reshape(1, 4, 1)
        jj = np.arange(64).reshape(1, 1, 64)
        for TT in range(2):
            for gr in range(4):
                g_abs = gr if TT == 0 else gr + 4
                ch = 4 * g_abs + kc
                j = 64 * gr + jj
                vis = (128 * ch + i) <= (2048 * TT + 8 * j + c)
                mask[:, 4 * TT + gr, :] = vis.reshape(128, 256)
        mask = mask.astype(bf)

        in_maps.append({
            "tok_t": tok_t_full,
            "tok_q": tok_q,
            "w_q": w_q,
            "b_q": b_q,
            "w_kv": w_kv,
            "b_k": b_k,
            "b_v": b_v,
            "w_p": w_p,
            "b_p": b_p,
            "maskp": mask,
            "onesBPp": onesBP,
            "ones128p": ones128,
            "ones1p": ones1,
        })
    return in_maps


def kernel(tokens, attn_bias, c_attn_weight, c_attn_bias, c_proj_weight,
           c_proj_bias):
    if "nc" not in _CACHE:
        _CACHE["nc"] = build_bass()
    nc = _CACHE["nc"]
    in_maps = make_inputs_v2(
        tokens, c_attn_weight, c_attn_bias, c_proj_weight, c_proj_bias
    )
    res = run_bass_kernel_spmd(nc, in_maps, list(range(NC)), trace=TRACE)
    _CACHE["last_result"] = res
    out = np.zeros((S, D), np.float32)
    for c in range(NC):
        out[np.arange(QL) * NC + c] = res.results[c]["out"]
    return out
